# revision 26
# baseline (speedup 1.0000x reference)
"""Trainium2 Bass kernel for nn_Canny: 5x5 Gaussian blur -> Sobel -> channel
argmax -> directional NMS -> threshold+sigmoid, on 16x3x512x512, data-parallel
across 8 NeuronCores (2 images each).

v2 redesign vs the chunked baseline:
  - 4 NON-overlapping column chunks (was 5 overlapping); cross-chunk conv and
    NMS halos are patched with tiny PSUM-accumulating edge matmuls on PE.
  - Element-wise NMS/select/sigmoid phase runs IMAGE-WIDE ([128, 2048] tiles)
    instead of per-chunk, amortizing per-op engine overheads 4x.
  - Fused custom DVE ops: M2 = gx^2+gy^2 and QX = gx^2*sign(gx*gy) (sign of
    the diagonal bin is carried in QX's sign bit), so only ONE cross-channel
    select chain is needed.
  - Direction bins on the selected channel: ver <=> |qx| >= k3*mag2,
    hor <=> |qx| < k1*mag2 with k = T^2/(1+T^2) (algebraically equal to the
    baseline's formulation on the same bf16 inputs).
  - Comparisons/maxes/copies are split across DVE, ACT (scalar) and the
    otherwise-idle GPSIMD (Pool) engine to balance engine busy time.

Output is produced transposed per image ([col, row]) for contiguous DMA; the
host swaps axes back. Borders (<=4 px) use zero-pad approximations of the
reference's mixed zero/replicate padding; interior matches to bf16 rounding.
"""

from contextlib import ExitStack

import numpy as np
import ml_dtypes

import concourse.bacc as bacc
import concourse.tile as tile
import concourse.mybir as mybir

F32 = mybir.dt.float32
BF16 = mybir.dt.bfloat16
U16 = mybir.dt.uint16
ALU = mybir.AluOpType
ACTF = mybir.ActivationFunctionType


def _register_op(name, make_spec):
    """Register a custom DVE op with runtime-computed table sha."""
    import concourse.dve_ops as dvo
    from concourse.dve_spec import lower, _has_src1
    from concourse.dve_uop import DveOpSpec

    if name in dvo._SUB_OPCODE_FOR_NAME:
        return next(op for op in dvo.OPS if op.name == name)
    spec = make_spec()
    opcode = dvo._CUSTOM_DVE_ROW_BASE + len(dvo.OPS)
    dvo._SUB_OPCODE_FOR_NAME[name] = opcode
    shas = {}
    for ver in ("v3", "v4"):
        try:
            tmp = DveOpSpec(name=name, opcode=opcode, uops=lower(spec, ver=ver),
                            rd1_en=_has_src1(spec))
            shas[ver] = tmp.sha(ver)
        except Exception:
            pass
    op = dvo.DveOp(name, spec, subdim=False, uops_sha=shas)
    dvo.OPS.append(op)
    dvo.CUSTOM_DVE_SPECS[name] = spec
    return op


# out = (in0*(s0 + s1*in0^2))*in1 + imm2 : masked minimax cubic for
# sigmoid(m)-0.5 on m in [0, 0.78], rebased at +0.5. keep==0 -> exactly 0.5.
def _sigpoly_spec():
    from concourse.dve_spec import Spec, Src0, Src1, C0, C1, C2

    return Spec(
        body=(Src0 * (C0 + C1 * (Src0 * Src0))) * Src1 + C2,
        reference=lambda in0, in1, s0, s1, imm2: (
            in0.astype(np.float32) * (s0 + s1 * in0.astype(np.float32) ** 2)
        ) * in1 + imm2,
    )


def _m2_spec():
    from concourse.dve_spec import Spec, Src0, Src1

    return Spec(
        body=Src0 * Src0 + Src1 * Src1,
        reference=lambda in0, in1, s0, s1, imm2: (
            in0.astype(np.float32) ** 2 + in1.astype(np.float32) ** 2),
    )


# qx = gx^2 * sign(gx*gy): |qx| is the selected-channel gx^2, sign(qx) picks
# the diagonal bin (d1 when >= 0).
def _qx_spec():
    from concourse.dve_spec import Spec, Src0, Src1, C0, C1, select

    return Spec(
        body=select(Src0 * Src1 >= C0, Src0 * Src0, C1 - Src0 * Src0),
        reference=lambda in0, in1, s0, s1, imm2: np.where(
            in0.astype(np.float32) * in1 >= s0,
            in0.astype(np.float32) ** 2, s1 - in0.astype(np.float32) ** 2),
    )


# ver: |qx| >= s0*mag2 ; hor: s0*mag2 > |qx|  (s0 = T^2/(1+T^2))
def _ver2_spec():
    from concourse.dve_spec import Spec, Src0, Src1, C0, maxx

    return Spec(
        body=maxx(Src0, -Src0) >= C0 * Src1,
        reference=lambda in0, in1, s0, s1, imm2: (
            np.abs(in0.astype(np.float32)) >= s0 * in1).astype(np.float32),
    )


def _hor2_spec():
    from concourse.dve_spec import Spec, Src0, Src1, C0, maxx

    return Spec(
        body=C0 * Src1 > maxx(Src0, -Src0),
        reference=lambda in0, in1, s0, s1, imm2: (
            s0 * in1 > np.abs(in0.astype(np.float32))).astype(np.float32),
    )


_SIGPOLY = _register_op("SIGMASK_POLY_ANT", _sigpoly_spec)
_M2 = _register_op("M2_SUMSQ_ANT", _m2_spec)
_QX = _register_op("QX_SIGNSQ_ANT", _qx_spec)
_VER2 = _register_op("VER_BIN2_ANT", _ver2_spec)
_HOR2 = _register_op("HOR_BIN2_ANT", _hor2_spec)


def _sig_coefs():
    # least-squares odd cubic fit of sigmoid(m)-0.5 over the reachable range
    m = np.linspace(0, 0.78, 2001)
    y = 1.0 / (1.0 + np.exp(-m)) - 0.5
    A = np.stack([m, m ** 3], axis=1)
    c, *_ = np.linalg.lstsq(A, y, rcond=None)
    return float(c[0]), float(c[1])


SIG_C1, SIG_C3 = _sig_coefs()

H = W = 512
C = 3
IMGS = 2          # images per core
N_CORES = 8
STARTS = [0, 120, 240, 360, 384]                      # stage-1 row blocks
RESP = [(0, 124), (125, 244), (245, 364), (365, 484), (485, 511)]
CCS = [0, 128, 256, 384]                              # column chunks
NCH = 4
SEG = 514                                             # 512 + 2 border cols
WID = NCH * 512                                       # wide free size
T1SQ = float(np.tan(np.pi / 8)) ** 2
T3SQ = float(np.tan(3 * np.pi / 8)) ** 2
K1 = T1SQ / (1.0 + T1SQ)
K3 = T3SQ / (1.0 + T3SQ)


def _np_consts():
    ax = np.arange(5) - 2.0
    g = np.exp(-(ax ** 2) / 2.0)
    g = g / g.sum()
    a7 = np.convolve(g, np.array([1.0, 2.0, 1.0])) / 2.0
    b7 = np.convolve(g, np.array([-1.0, 0.0, 1.0])) / 4.0

    def band1(taps, t):
        s = STARTS[t]
        r0, r1 = RESP[t]
        L = r1 - r0 + 1
        B = np.zeros((128, L), np.float32)
        for k in range(128):
            for j in range(L):
                d = (s + k) - (r0 + j) + 3
                if 0 <= d <= 6:
                    B[k, j] = taps[d]
        return B

    def band2(taps):
        B = np.zeros((128, 128), np.float32)
        for k in range(128):
            for j in range(128):
                d = (k - j) + 3
                if 0 <= d <= 6:
                    B[k, j] = taps[d]
        return B

    # Edge-correction matrices, padded to 32-aligned PE tile windows:
    # left corr reads prev chunk's top-3 partitions (125..127 of a [96:128]
    # window) and accumulates into out partitions 0..2 (of a [0:32] window);
    # right corr reads next chunk's partitions 0..2 and writes 125..127
    # (cols 29..31 of a [96:128] window).
    def bl(taps):
        B = np.zeros((128, 32), np.float32)
        for i in range(3):
            for j in range(3):
                if 0 <= i - j <= 6:
                    B[125 + i, j] = taps[i - j]
        return B

    def br(taps):
        B = np.zeros((128, 64), np.float32)
        for i in range(3):
            for j in range(3):
                if 0 <= 6 + i - j <= 6:
                    B[i, 61 + j] = taps[6 + i - j]
        return B

    SL = np.zeros((128, 128), np.float32)  # out[j] = in[j-1]
    SR = np.zeros((128, 128), np.float32)  # out[j] = in[j+1]
    for j in range(1, 128):
        SL[j - 1, j] = 1.0
    for j in range(0, 127):
        SR[j + 1, j] = 1.0
    MLC = np.zeros((128, 32), np.float32)  # out[0] += in[127] (prev chunk)
    MLC[127, 0] = 1.0
    MRC = np.zeros((128, 64), np.float32)  # out[127] += in[0] (next chunk)
    MRC[0, 63] = 1.0

    consts = {}
    for t in range(5):
        consts[f"b1u_{t}"] = band1(a7, t)
        consts[f"b1v_{t}"] = band1(b7, t)
    consts["b2gx"] = band2(b7)
    consts["b2gy"] = band2(a7)
    consts["blx"] = bl(b7)
    consts["brx"] = br(b7)
    consts["bly"] = bl(a7)
    consts["bry"] = br(a7)
    consts["SL"] = SL
    consts["SR"] = SR
    consts["mlc"] = MLC
    consts["mrc"] = MRC
    return consts


def _load_image(nc, pools, xin, b):
    """DMA + bf16-cast (on GPSIMD) all row tiles of image b."""
    pool_xf, pool_xbf = pools["xf"], pools["xbf"]
    xbf = {}
    for t in range(5):
        for c in range(C):
            xf = pool_xf.tile([128, W], F32, tag="xf")
            nc.sync.dma_start(out=xf, in_=xin[b, c, STARTS[t]:STARTS[t] + 128, :])
            xb = pool_xbf.tile([128, W], BF16, tag=f"xbf_{t}_{c}")
            nc.gpsimd.tensor_copy(out=xb, in_=xf)
            xbf[t, c] = xb
    return xbf


def _emit_image(nc, pools, cb, xbf, yT, b, tsq):
    v = nc.vector
    a = nc.scalar
    g = nc.gpsimd
    ps_uv, ps_g = pools["psuv"], pools["psg"]
    wp = pools["wide"]

    # ---- stage 1: vertical convs (transposing matmuls), all chunks ----
    uvb = {}
    for cc in range(NCH):
        cs = CCS[cc]
        for c in range(C):
            uvps = ps_uv.tile([128, 2 * W], F32, tag="uv")
            ups = uvps[:, 0:W]
            vps = uvps[:, W:2 * W]
            for t in range(5):
                r0, r1 = RESP[t]
                L = r1 - r0 + 1
                lhsT = xbf[t, c][:, cs:cs + 128]
                nc.tensor.matmul(ups[:, r0:r1 + 1], lhsT, cb[f"b1u_{t}"][:, :L],
                                 start=True, stop=True)
                nc.tensor.matmul(vps[:, r0:r1 + 1], lhsT, cb[f"b1v_{t}"][:, :L],
                                 start=True, stop=True)
            ub = pools["uvb"].tile([128, 2 * W], BF16, tag=f"uvb_{cc}_{c}")
            a.copy(out=ub, in_=uvps)
            uvb[cc, c] = ub

    # wide per-channel tiles
    m2w = [wp.tile([128, WID], BF16, tag=f"m2w{c}", name=f"m2w{c}")
           for c in range(C)]
    qxw = [wp.tile([128, WID], BF16, tag=f"qxw{c}", name=f"qxw{c}")
           for c in range(C)]

    # ---- stage 2: horizontal convs + fused square/sign customs ----
    for cc in range(NCH):
        for c in range(C):
            gxy = ps_g.tile([128, 2 * W], F32, tag="g")
            gx = gxy[:, 0:W]
            gy = gxy[:, W:2 * W]
            u = uvb[cc, c][:, 0:W]
            vv = uvb[cc, c][:, W:2 * W]
            for (dst, band, bli, bri, src) in (
                    (gx, "b2gx", "blx", "brx", 0), (gy, "b2gy", "bly", "bry", W)):
                writers = []
                if cc > 0:
                    pu = uvb[cc - 1, c][:, src:src + W]
                    writers.append((dst[0:32, :], cb[bli][64:128, :],
                                    pu[64:128, :]))
                if cc < NCH - 1:
                    nu = uvb[cc + 1, c][:, src:src + W]
                    writers.append((dst[64:128, :], cb[bri][0:32, :],
                                    nu[0:32, :]))
                rhs = u if src == 0 else vv
                nc.tensor.matmul(dst, cb[band], rhs, start=True,
                                 stop=(not writers))
                for i, (o, l, r) in enumerate(writers):
                    nc.tensor.matmul(o, l, r, start=False,
                                     stop=(i == len(writers) - 1),
                                     skip_group_check=True)
            # custom DVE ops may read only one PSUM operand: stage gy in SBUF
            gyb = pools["gyb"].tile([128, W], BF16, tag="gyb")
            a.copy(out=gyb, in_=gy)
            sl = slice(cc * W, (cc + 1) * W)
            v._custom_dve(_M2, out=m2w[c][:, sl], in0=gx, in1=gyb,
                          s0=0.0, s1=0.0)
            v._custom_dve(_QX, out=qxw[c][:, sl], in0=gx, in1=gyb,
                          s0=0.0, s1=0.0)

    # ---- wide phase: argmax select, bins, NMS, sigmoid ----
    r3 = lambda t: t.rearrange("p (c w) -> p c w", c=NCH)
    magb = wp.tile([128, NCH * SEG], BF16, tag="magb")
    magb3 = r3v = magb.rearrange("p (c w) -> p c w", c=NCH)
    v.memset(magb3[:, :, 0:1], 0.0)
    v.memset(magb3[:, :, SEG - 1:SEG], 0.0)
    magc = magb3[:, :, 1:W + 1]

    mx01 = wp.tile([128, WID], BF16, tag="mx01")
    v.tensor_max(mx01, m2w[0], m2w[1])
    v.tensor_max(magc, r3(mx01), r3(m2w[2]))

    eq0 = wp.tile([128, WID], U16, tag="eq0")
    eq1 = wp.tile([128, WID], U16, tag="eq1")
    v.tensor_tensor(r3(eq0), r3(m2w[0]), magc, ALU.is_equal)
    v.tensor_tensor(r3(eq1), r3(m2w[1]), magc, ALU.is_equal)

    qxs = wp.tile([128, WID], BF16, tag="qxs")
    v.tensor_copy(qxs, qxw[2])
    v.copy_predicated(out=qxs, mask=eq1, data=qxw[1])
    v.copy_predicated(out=qxs, mask=eq0, data=qxw[0])

    ver = wp.tile([128, WID], U16, tag="ver")
    hor = wp.tile([128, WID], U16, tag="hor")
    v._custom_dve(_VER2, out=r3(ver), in0=r3(qxs), in1=magc, s0=K3, s1=0.0)
    v._custom_dve(_HOR2, out=r3(hor), in0=r3(qxs), in1=magc, s0=K1, s1=0.0)
    # negq = relu(-qxs): nonzero exactly where the diagonal sign picks d2
    negq = wp.tile([128, WID], BF16, tag="negq")
    a.activation(out=negq, in_=qxs, func=ACTF.Relu, scale=-1.0)

    # ---- NMS column-neighbours via shift matmuls (+ cross-chunk patches) ----
    mlr = wp.tile([128, 2 * NCH * SEG], BF16, tag="mlr")
    mlr4 = mlr.rearrange("p (a c w) -> p a c w", a=2, c=NCH)
    g.memset(mlr4[:, :, :, 0:1], 0.0)
    g.memset(mlr4[:, :, :, SEG - 1:SEG], 0.0)
    for cc in range(NCH):
        mLR = ps_uv.tile([128, 2 * W], F32, tag="uv")
        mseg = magb[:, cc * SEG + 1:cc * SEG + 1 + W]
        nc.tensor.matmul(mLR[:, 0:W], cb["SL"], mseg, start=True,
                         stop=(cc == 0))
        if cc > 0:
            pseg = magb[:, (cc - 1) * SEG + 1:(cc - 1) * SEG + 1 + W]
            nc.tensor.matmul(mLR[0:32, 0:W], cb["mlc"][64:128, :],
                             pseg[64:128, :], start=False, stop=True,
                             skip_group_check=True)
        nc.tensor.matmul(mLR[:, W:2 * W], cb["SR"], mseg, start=True,
                         stop=(cc == NCH - 1))
        if cc < NCH - 1:
            nseg = magb[:, (cc + 1) * SEG + 1:(cc + 1) * SEG + 1 + W]
            nc.tensor.matmul(mLR[64:128, W:2 * W], cb["mrc"][0:32, :],
                             nseg[0:32, :], start=False, stop=True,
                             skip_group_check=True)
        a.activation(out=mlr4[:, :, cc, 1:W + 1],
                     in_=mLR.rearrange("p (a w) -> p a w", a=2),
                     func=ACTF.Copy)

    mL = mlr4[:, 0]
    mR = mlr4[:, 1]
    horn = wp.tile([128, WID], BF16, tag="horn")
    vern = wp.tile([128, WID], BF16, tag="vern")
    d1n = wp.tile([128, WID], BF16, tag="d1n")
    d2n = wp.tile([128, WID], BF16, tag="d2n")
    v.tensor_max(r3(horn), magb3[:, :, 0:W], magb3[:, :, 2:W + 2])
    v.tensor_max(r3(vern), mL[:, :, 1:W + 1], mR[:, :, 1:W + 1])
    v.tensor_max(r3(d1n), mL[:, :, 0:W], mR[:, :, 2:W + 2])
    v.tensor_max(r3(d2n), mR[:, :, 0:W], mL[:, :, 2:W + 2])

    nbr = wp.tile([128, WID], BF16, tag="nbr")
    v.tensor_copy(nbr, d1n)
    v.copy_predicated(out=nbr, mask=negq.bitcast(U16), data=d2n)
    v.copy_predicated(out=nbr, mask=hor, data=horn)
    v.copy_predicated(out=nbr, mask=ver, data=vern)

    # keep = [max(nbr, t^2) <= magsq]
    keep = wp.tile([128, WID], BF16, tag="keep")
    v.scalar_tensor_tensor(r3(keep), r3(nbr), tsq, magc, ALU.max, ALU.is_le)

    # ---- sigmoid(mag) via fused masked cubic, rebased at 0.5 ----
    sqr = wp.tile([128, WID], BF16, tag="sqr")
    a.activation(out=r3(sqr), in_=magc, func=ACTF.Sqrt, bias=cb["eps9"][:, 0:1])
    outf = wp.tile([128, WID], F32, tag="outf")
    v._custom_dve(_SIGPOLY, out=outf, in0=sqr, in1=keep,
                  s0=SIG_C1, s1=SIG_C3, imm2=0.5)

    for cc in range(NCH):
        cs = CCS[cc]
        nc.sync.dma_start(out=yT[b, cs:cs + 128, :],
                          in_=outf[:, cc * W:(cc + 1) * W])


def build_nc(tsq: float, repeat: int = 1):
    nc = bacc.Bacc("TRN2", debug=False, num_devices=N_CORES)
    xin = nc.dram_tensor("x", [IMGS, C, H, W], F32, kind="ExternalInput").ap()
    yT = nc.dram_tensor("yT", [IMGS, W, H], F32, kind="ExternalOutput").ap()

    consts = _np_consts()
    cdram = {k: nc.inline_tensor(a.astype(ml_dtypes.bfloat16), name=k).ap()
             for k, a in consts.items()}

    with tile.TileContext(nc) as tc, ExitStack() as ctx:
        cpool = ctx.enter_context(tc.tile_pool(name="consts", bufs=1))
        cb = {}
        for k, arr in consts.items():
            t = cpool.tile(list(arr.shape), BF16, tag=k)
            nc.sync.dma_start(out=t, in_=cdram[k])
            cb[k] = t
        eps9 = cpool.tile([128, 1], F32, tag="eps9")
        nc.vector.memset(eps9, 1e-9)
        cb["eps9"] = eps9
        pools = {
            "xf": ctx.enter_context(tc.tile_pool(name="xf", bufs=4)),
            "xbf": ctx.enter_context(tc.tile_pool(name="xbf", bufs=1)),
            "uvb": ctx.enter_context(tc.tile_pool(name="uvb", bufs=1)),
            "gyb": ctx.enter_context(tc.tile_pool(name="gyb", bufs=3)),
            "wide": ctx.enter_context(tc.tile_pool(name="wide", bufs=1)),
            "psuv": ctx.enter_context(tc.tile_pool(name="psuv", bufs=2,
                                                   space="PSUM")),
            "psg": ctx.enter_context(tc.tile_pool(name="psg", bufs=2,
                                                  space="PSUM")),
        }
        for _ in range(repeat):
            for b in range(IMGS):
                xbf = _load_image(nc, pools, xin, b)
                _emit_image(nc, pools, cb, xbf, yT, b, tsq)
    nc.compile()
    return nc


_cache = {}


def _get_nc(tsq: float):
    if tsq not in _cache:
        _cache[tsq] = build_nc(tsq)
    return _cache[tsq]


def kernel(x, low_threshold):
    from concourse.bass_utils import run_bass_kernel_spmd

    x = np.asarray(x, dtype=np.float32)
    t = float(np.asarray(low_threshold))
    nc = _get_nc(t * t)
    in_maps = [{"x": np.ascontiguousarray(x[IMGS * i:IMGS * (i + 1)])}
               for i in range(N_CORES)]
    res = run_bass_kernel_spmd(nc, in_maps, core_ids=list(range(N_CORES)))
    outT = np.stack([r["yT"] for r in res.results])  # [8, 2, W(col), H(row)]
    out = outT.reshape(N_CORES * IMGS, W, H).transpose(0, 2, 1)
    return np.ascontiguousarray(out).astype(np.float32, copy=False)
